# revision 13
# baseline (speedup 1.0000x reference)
"""Trainium2 Bass kernel for nn_KerasMultiLIFLayerSparseCell.

3 independent LIF layer steps. Sharding: tensor-parallel over output
neurons (W rows). Each of the 8 cores computes syn = C @ W_chunk.T for
its out-slice via PE matmuls, updates the membrane slice, computes the
spike mask and a local prefix-scan, then cross-core rank assembly:
  - AllGather of per-(core,batch) true-counts (tiny) -> global ranks
  - gpsimd local_scatter compacts indices by rank (top-k semantics of
    jax.lax.top_k on a 0/1 mask: true indices ascending, then false
    indices ascending)
  - ReduceScatter(add) merges the per-core partial id lists.
"""

import numpy as np

import concourse.bass as bass
import concourse.bacc as bacc
import concourse.mybir as mybir
import concourse.tile as tile
from concourse.bass_utils import run_bass_kernel_spmd

AL = mybir.AluOpType
F32 = mybir.dt.float32
I32 = mybir.dt.int32
I16 = mybir.dt.int16
U8 = mybir.dt.uint8

NC = 8
B = 128
BL = B // NC  # 16 batches per core after ReduceScatter
INS = [1024, 2048, 2048]   # contraction dims (W cols)
OUTS = [2048, 2048, 1024]  # output-neuron dims (W rows)
KS = [256, 256, 128]       # sparse_out k per layer
OL = [o // NC for o in OUTS]   # per-core out slice: 256, 256, 128
KT = [i // 128 for i in INS]   # K tiles: 8, 16, 16
CSEG = [0, 256, 512]           # chunk / k-segment bases (both sum to 640)
W_TOT = sum(OL)                # 640

DECAY = 0.95
THRESH = 1.0


def _build_program(stage=5):
    nc = bacc.Bacc()
    inp = {}
    for l in range(3):
        inp[f"mm{l}"] = nc.declare_dram_parameter(f"mm{l}", [INS[l], B + OL[l]], F32, isOutput=False)
        inp[f"vc{l}"] = nc.declare_dram_parameter(f"vc{l}", [B, OL[l]], F32, isOutput=False)
    inp["iota16"] = nc.declare_dram_parameter("iota16", [B, W_TOT], I16, isOutput=False)
    inp["iotaf"] = nc.declare_dram_parameter("iotaf", [B, W_TOT], F32, isOutput=False)
    inp["coremask"] = nc.declare_dram_parameter("coremask", [B, NC], F32, isOutput=False)
    out_vn = [nc.declare_dram_parameter(f"vn{l}", [B, OL[l]], F32, isOutput=True) for l in range(3)]
    out_ids = nc.declare_dram_parameter("ids_out", [BL, W_TOT], I32, isOutput=True)
    out_num = nc.declare_dram_parameter("num_out", [B, 3], I32, isOutput=True)

    with tile.TileContext(nc) as tc, \
            tc.tile_pool(name="sbuf", bufs=1) as _sb_pool, \
            tc.tile_pool(name="psum", bufs=1, space="PSUM") as _ps_pool, \
            tc.tile_pool(name="dram", bufs=1, space="DRAM") as _dr_pool:
        def T(shape, dtype, name, space="SBUF", addr_space="Local"):
            if space == "PSUM":
                return _ps_pool.tile(shape, dtype, name=name)
            if space == "DRAM":
                return _dr_pool.tile(shape, dtype, name=name, addr_space=addr_space)
            return _sb_pool.tile(shape, dtype, name=name)

        iota16_sb = T([B, W_TOT], I16, name="iota16_sb")
        nc.sync.dma_start(iota16_sb[:], inp["iota16"][:])
        iotaf_sb = T([B, W_TOT], F32, name="iotaf_sb")
        nc.sync.dma_start(iotaf_sb[:], inp["iotaf"][:])
        cmask_sb = T([B, NC], F32, name="cmask_sb")
        nc.sync.dma_start(cmask_sb[:], inp["coremask"][:])
        tot = T([B, 3], F32, name="tot")
        idx16 = T([B, W_TOT], I16, name="idx16")

        m01, cml = [], []
        for l in range(3):
            ps = T([128, OL[l]], F32, space="PSUM", name=f"ps{l}")
            for kt in range(KT[l]):
                mm_t = T([128, B + OL[l]], F32, name=f"mm{l}_{kt}")
                nc.sync.dma_start(mm_t[:], inp[f"mm{l}"][kt * 128:(kt + 1) * 128, :])
                nc.tensor.matmul(ps[:], mm_t[:, 0:B], mm_t[:, B:B + OL[l]],
                                 start=(kt == 0), stop=(kt == KT[l] - 1))
            v = T([B, OL[l]], F32, name=f"v{l}")
            nc.sync.dma_start(v[:], inp[f"vc{l}"][:])
            fired = T([B, OL[l]], F32, name=f"fired{l}")
            nc.vector.tensor_scalar(fired[:], v[:], THRESH, None, AL.is_ge)
            vt = T([B, OL[l]], F32, name=f"vt{l}")
            nc.vector.tensor_tensor(vt[:], v[:], fired[:], AL.mult)
            vr = T([B, OL[l]], F32, name=f"vr{l}")
            nc.vector.tensor_sub(vr[:], v[:], vt[:])
            vd = T([B, OL[l]], F32, name=f"vd{l}")
            nc.vector.tensor_scalar_mul(vd[:], vr[:], DECAY)
            vn = T([B, OL[l]], F32, name=f"vn{l}sb")
            nc.vector.scalar_tensor_tensor(vn[:], ps[:], 1.0 - DECAY, vd[:], AL.mult, AL.add)
            nc.sync.dma_start(out_vn[l][:], vn[:])
            m = T([B, OL[l]], U8, name=f"m{l}")
            nc.vector.tensor_scalar(m[:], vn[:], THRESH, None, AL.is_gt)
            cm = T([B, OL[l]], F32, name=f"cm{l}")
            nc.vector.tensor_tensor_scan(cm[:], m[:], m[:], 0.0, AL.add, AL.bypass)
            nc.vector.tensor_copy(tot[:, l:l + 1], cm[:, OL[l] - 1:OL[l]])
            m01.append(m)
            cml.append(cm)

        # cross-core exchange of true-counts
        ag_in = T([B, 3], F32, space="DRAM", name="ag_in")
        ag_out = T([B * NC, 3], F32, space="DRAM", addr_space="Shared", name="ag_out")
        nc.gpsimd.dma_start(ag_in[:], tot[:])
        if stage & 1:
            nc.gpsimd.collective_compute(
                "AllGather", AL.bypass, replica_groups=[list(range(NC))],
                ins=[ag_in[:]], outs=[ag_out[:]])
            totg = T([B, NC, 3], F32, name="totg")
            nc.gpsimd.dma_start(totg[:], ag_out.rearrange("(c b) l -> b c l", b=B))
        else:
            totg = T([B, NC, 3], F32, name="totg")
            nc.vector.memset(totg[:], 1.0)

        Tg, offs = [], []
        for l in range(3):
            tg8 = T([B, NC], F32, name=f"tg8_{l}")
            nc.vector.tensor_copy(tg8[:], totg[:, :, l:l + 1])
            ms = T([B, NC], F32, name=f"ms{l}")
            nc.vector.tensor_tensor(ms[:], tg8[:], cmask_sb[:], AL.mult)
            of = T([B, 1], F32, name=f"of{l}")
            nc.vector.tensor_reduce(of[:], ms[:], mybir.AxisListType.X, AL.add)
            tg = T([B, 1], F32, name=f"tg{l}")
            nc.vector.tensor_reduce(tg[:], tg8[:], mybir.AxisListType.X, AL.add)
            Tg.append(tg)
            offs.append(of)

        for l in range(3):
            om1 = T([B, 1], F32, name=f"om1_{l}")
            nc.vector.tensor_scalar_sub(om1[:], offs[l][:], 1.0)
            cmg = T([B, OL[l]], F32, name=f"cmg{l}")
            nc.vector.scalar_tensor_tensor(cmg[:], cml[l][:], offs[l][:], cml[l][:], AL.add, AL.bypass)
            rt = T([B, OL[l]], F32, name=f"rt{l}")
            nc.vector.scalar_tensor_tensor(rt[:], cml[l][:], om1[:], cml[l][:], AL.add, AL.bypass)
            rf = T([B, OL[l]], F32, name=f"rf{l}")
            nc.vector.scalar_tensor_tensor(
                rf[:], iotaf_sb[:, CSEG[l]:CSEG[l] + OL[l]], Tg[l][:], cmg[:], AL.add, AL.subtract)
            r0 = T([B, OL[l]], F32, name=f"r0{l}")
            nc.vector.tensor_copy(r0[:], rf[:])
            nc.vector.copy_predicated(r0[:], m01[l][:], rt[:])
            pred = T([B, OL[l]], F32, name=f"pred{l}")
            nc.vector.tensor_scalar(pred[:], r0[:], float(KS[l]), None, AL.is_ge)
            ixf = T([B, OL[l]], F32, name=f"ixf{l}")
            nc.vector.scalar_tensor_tensor(ixf[:], pred[:], -8192.0, r0[:], AL.mult, AL.add)
            ixf2 = T([B, OL[l]], F32, name=f"ixf2_{l}")
            nc.vector.tensor_scalar_add(ixf2[:], ixf[:], float(CSEG[l]))
            nc.vector.tensor_copy(idx16[:, CSEG[l]:CSEG[l] + OL[l]], ixf2[:])

        dst16 = T([B, W_TOT], I16, name="dst16")
        if not (stage & 2):
            nc.vector.memset(dst16[:], 0)
        else:
            nc.gpsimd.local_scatter(dst16[:], iota16_sb[:], idx16[:],
                                    channels=128, num_elems=W_TOT, num_idxs=W_TOT)
        dstf = T([B, W_TOT], F32, name="dstf")
        nc.vector.tensor_copy(dstf[:], dst16[:])
        rs_in = T([B, W_TOT], F32, space="DRAM", name="rs_in")
        rs_out = T([BL, W_TOT], F32, space="DRAM", name="rs_out")
        nc.gpsimd.dma_start(rs_in[:], dstf[:])
        if stage & 4:
            nc.gpsimd.collective_compute(
                "ReduceScatter", AL.add, replica_groups=[list(range(NC))],
                ins=[rs_in[:]], outs=[rs_out[:]])
        else:
            nc.gpsimd.dma_start(rs_out[:], rs_in[0:BL, :])
        ids_sb = T([BL, W_TOT], F32, name="ids_sb")
        nc.gpsimd.dma_start(ids_sb[:], rs_out[:])
        ids32 = T([BL, W_TOT], I32, name="ids32")
        nc.vector.tensor_copy(ids32[:], ids_sb[:])
        nc.sync.dma_start(out_ids[:], ids32[:])

        nums = T([B, 3], F32, name="nums")
        for l in range(3):
            nc.vector.tensor_scalar_min(nums[:, l:l + 1], Tg[l][:], float(KS[l]))
        numi = T([B, 3], I32, name="numi")
        nc.vector.tensor_copy(numi[:], nums[:])
        nc.sync.dma_start(out_num[:], numi[:])
    nc.finalize()
    return nc


_PROG = {}


def _get_prog():
    import os
    stage = int(os.environ.get("KSTAGE", "7"))
    if stage not in _PROG:
        _PROG[stage] = _build_program(stage)
    return _PROG[stage]


def _build_C(ids, num, in_dim):
    b, s = ids.shape
    C = np.zeros((b, in_dim), np.float32)
    maskf = (np.arange(s)[None, :] < num).astype(np.float32)
    np.add.at(C, (np.arange(b)[:, None], ids.astype(np.int64)), maskf)
    return C


def _make_in_maps(inputs):
    w = [np.asarray(inputs["w0"], np.float32), np.asarray(inputs["w1"], np.float32),
         np.asarray(inputs["w2"], np.float32)]
    v = [np.asarray(inputs["v0"], np.float32), np.asarray(inputs["v1"], np.float32),
         np.asarray(inputs["v2"], np.float32)]
    C = [
        _build_C(np.asarray(inputs["inp_ids"]), np.asarray(inputs["inp_num"]), INS[0]),
        _build_C(np.asarray(inputs["s1_ids"]), np.asarray(inputs["s1_num"]), INS[1]),
        _build_C(np.asarray(inputs["s2_ids"]), np.asarray(inputs["s2_num"]), INS[2]),
    ]
    ct = [np.ascontiguousarray(c.T) for c in C]
    in_maps = []
    for c in range(NC):
        m = {}
        for l in range(3):
            ol = OL[l]
            wt_c = w[l][c * ol:(c + 1) * ol, :].T
            m[f"mm{l}"] = np.ascontiguousarray(np.concatenate([ct[l], wt_c], axis=1))
            m[f"vc{l}"] = np.ascontiguousarray(v[l][:, c * ol:(c + 1) * ol])
        iota = np.concatenate([c * OL[l] + np.arange(OL[l]) for l in range(3)])
        m["iota16"] = np.broadcast_to(iota.astype(np.int16), (B, W_TOT)).copy()
        m["iotaf"] = np.broadcast_to(iota.astype(np.float32), (B, W_TOT)).copy()
        cmsk = np.zeros((B, NC), np.float32)
        cmsk[:, :c] = 1.0
        m["coremask"] = cmsk
        in_maps.append(m)
    return in_maps


def _assemble(r):
    ids = np.concatenate([r[c]["ids_out"] for c in range(NC)], axis=0).astype(np.int32)
    num3 = r[0]["num_out"].astype(np.int32)
    vns = [np.concatenate([r[c][f"vn{l}"] for c in range(NC)], axis=1) for l in range(3)]
    return (ids[:, 0:256], num3[:, 0:1], ids[:, 256:512], num3[:, 1:2],
            ids[:, 512:640], num3[:, 2:3], vns[0], vns[1], vns[2])


def _run(inputs, trace=False):
    in_maps = _make_in_maps(inputs)
    res = run_bass_kernel_spmd(_get_prog(), in_maps, list(range(NC)), trace=trace)
    out = _assemble(res.results)
    return out, res


def kernel(**inputs):
    out, _ = _run(inputs, trace=False)
    return out


# revision 14
# speedup vs baseline: 1.0828x; 1.0828x over previous
"""Trainium2 Bass kernel for nn_KerasMultiLIFLayerSparseCell.

3 independent LIF layer steps. Sharding: tensor-parallel over output
neurons (W rows). Each of the 8 cores computes syn = C @ W_chunk.T for
its out-slice via PE matmuls, updates the membrane slice, computes the
spike mask and a local prefix-scan, then cross-core rank assembly:
  - AllGather of per-(core,batch) true-counts (tiny) -> global ranks
  - gpsimd local_scatter compacts indices by rank (top-k semantics of
    jax.lax.top_k on a 0/1 mask: true indices ascending, then false
    indices ascending)
  - ReduceScatter(add) merges the per-core partial id lists.
"""

import numpy as np

import concourse.bass as bass
import concourse.bacc as bacc
import concourse.mybir as mybir
import concourse.tile as tile
from concourse.bass_utils import run_bass_kernel_spmd

AL = mybir.AluOpType
F32 = mybir.dt.float32
I32 = mybir.dt.int32
I16 = mybir.dt.int16
U8 = mybir.dt.uint8

NC = 8
B = 128
BL = B // NC  # 16 batches per core after ReduceScatter
INS = [1024, 2048, 2048]   # contraction dims (W cols)
OUTS = [2048, 2048, 1024]  # output-neuron dims (W rows)
KS = [256, 256, 128]       # sparse_out k per layer
OL = [o // NC for o in OUTS]   # per-core out slice: 256, 256, 128
KT = [i // 128 for i in INS]   # K tiles: 8, 16, 16
CSEG = [0, 256, 512]           # chunk / k-segment bases (both sum to 640)
W_TOT = sum(OL)                # 640

DECAY = 0.95
THRESH = 1.0


def _build_program(stage=5):
    nc = bacc.Bacc()
    inp = {}
    for l in range(3):
        inp[f"ct{l}"] = nc.declare_dram_parameter(f"ct{l}", [INS[l], B], U8, isOutput=False)
        inp[f"wt{l}"] = nc.declare_dram_parameter(f"wt{l}", [INS[l], OL[l]], F32, isOutput=False)
        inp[f"vc{l}"] = nc.declare_dram_parameter(f"vc{l}", [B, OL[l]], F32, isOutput=False)
    inp["iota16"] = nc.declare_dram_parameter("iota16", [B, W_TOT], I16, isOutput=False)
    inp["iotaf"] = nc.declare_dram_parameter("iotaf", [B, W_TOT], F32, isOutput=False)
    inp["coremask"] = nc.declare_dram_parameter("coremask", [B, NC], F32, isOutput=False)
    out_vn = [nc.declare_dram_parameter(f"vn{l}", [B, OL[l]], F32, isOutput=True) for l in range(3)]
    out_ids = nc.declare_dram_parameter("ids_out", [BL, W_TOT], I32, isOutput=True)
    out_num = nc.declare_dram_parameter("num_out", [B, 3], I32, isOutput=True)

    with tile.TileContext(nc) as tc, \
            tc.tile_pool(name="sbuf", bufs=1) as _sb_pool, \
            tc.tile_pool(name="psum", bufs=1, space="PSUM") as _ps_pool, \
            tc.tile_pool(name="dram", bufs=1, space="DRAM") as _dr_pool:
        def T(shape, dtype, name, space="SBUF", addr_space="Local"):
            if space == "PSUM":
                return _ps_pool.tile(shape, dtype, name=name)
            if space == "DRAM":
                return _dr_pool.tile(shape, dtype, name=name, addr_space=addr_space)
            return _sb_pool.tile(shape, dtype, name=name)

        iota16_sb = T([B, W_TOT], I16, name="iota16_sb")
        nc.sync.dma_start(iota16_sb[:], inp["iota16"][:])
        iotaf_sb = T([B, W_TOT], F32, name="iotaf_sb")
        nc.sync.dma_start(iotaf_sb[:], inp["iotaf"][:])
        cmask_sb = T([B, NC], F32, name="cmask_sb")
        nc.sync.dma_start(cmask_sb[:], inp["coremask"][:])
        tot = T([B, 3], F32, name="tot")
        idx16 = T([B, W_TOT], I16, name="idx16")

        m01, cml = [], []
        for l in range(3):
            ps = T([128, OL[l]], F32, space="PSUM", name=f"ps{l}")
            for kt in range(KT[l]):
                ctu = T([128, B], U8, name=f"ctu{l}_{kt}")
                nc.sync.dma_start(ctu[:], inp[f"ct{l}"][kt * 128:(kt + 1) * 128, :])
                ctf = T([128, B], F32, name=f"ctf{l}_{kt}")
                nc.gpsimd.tensor_copy(ctf[:], ctu[:])
                wt_t = T([128, OL[l]], F32, name=f"wt{l}_{kt}")
                nc.sync.dma_start(wt_t[:], inp[f"wt{l}"][kt * 128:(kt + 1) * 128, :])
                nc.tensor.matmul(ps[:], ctf[:], wt_t[:],
                                 start=(kt == 0), stop=(kt == KT[l] - 1))
            v = T([B, OL[l]], F32, name=f"v{l}")
            nc.sync.dma_start(v[:], inp[f"vc{l}"][:])
            fired = T([B, OL[l]], F32, name=f"fired{l}")
            nc.vector.tensor_scalar(fired[:], v[:], THRESH, None, AL.is_ge)
            vt = T([B, OL[l]], F32, name=f"vt{l}")
            nc.vector.tensor_tensor(vt[:], v[:], fired[:], AL.mult)
            vr = T([B, OL[l]], F32, name=f"vr{l}")
            nc.vector.tensor_sub(vr[:], v[:], vt[:])
            vd = T([B, OL[l]], F32, name=f"vd{l}")
            nc.vector.tensor_scalar_mul(vd[:], vr[:], DECAY)
            vn = T([B, OL[l]], F32, name=f"vn{l}sb")
            nc.vector.scalar_tensor_tensor(vn[:], ps[:], 1.0 - DECAY, vd[:], AL.mult, AL.add)
            nc.sync.dma_start(out_vn[l][:], vn[:])
            m = T([B, OL[l]], U8, name=f"m{l}")
            nc.vector.tensor_scalar(m[:], vn[:], THRESH, None, AL.is_gt)
            cm = T([B, OL[l]], F32, name=f"cm{l}")
            nc.vector.tensor_tensor_scan(cm[:], m[:], m[:], 0.0, AL.add, AL.bypass)
            nc.vector.tensor_copy(tot[:, l:l + 1], cm[:, OL[l] - 1:OL[l]])
            m01.append(m)
            cml.append(cm)

        # cross-core exchange of true-counts
        ag_in = T([B, 3], F32, space="DRAM", name="ag_in")
        ag_out = T([B * NC, 3], F32, space="DRAM", addr_space="Shared", name="ag_out")
        nc.gpsimd.dma_start(ag_in[:], tot[:])
        if stage & 1:
            nc.gpsimd.collective_compute(
                "AllGather", AL.bypass, replica_groups=[list(range(NC))],
                ins=[ag_in[:]], outs=[ag_out[:]])
            totg = T([B, NC, 3], F32, name="totg")
            nc.gpsimd.dma_start(totg[:], ag_out.rearrange("(c b) l -> b c l", b=B))
        else:
            totg = T([B, NC, 3], F32, name="totg")
            nc.vector.memset(totg[:], 1.0)

        Tg, offs = [], []
        for l in range(3):
            tg8 = T([B, NC], F32, name=f"tg8_{l}")
            nc.vector.tensor_copy(tg8[:], totg[:, :, l:l + 1])
            ms = T([B, NC], F32, name=f"ms{l}")
            nc.vector.tensor_tensor(ms[:], tg8[:], cmask_sb[:], AL.mult)
            of = T([B, 1], F32, name=f"of{l}")
            nc.vector.tensor_reduce(of[:], ms[:], mybir.AxisListType.X, AL.add)
            tg = T([B, 1], F32, name=f"tg{l}")
            nc.vector.tensor_reduce(tg[:], tg8[:], mybir.AxisListType.X, AL.add)
            Tg.append(tg)
            offs.append(of)

        for l in range(3):
            om1 = T([B, 1], F32, name=f"om1_{l}")
            nc.vector.tensor_scalar_sub(om1[:], offs[l][:], 1.0)
            cmg = T([B, OL[l]], F32, name=f"cmg{l}")
            nc.vector.scalar_tensor_tensor(cmg[:], cml[l][:], offs[l][:], cml[l][:], AL.add, AL.bypass)
            rt = T([B, OL[l]], F32, name=f"rt{l}")
            nc.vector.scalar_tensor_tensor(rt[:], cml[l][:], om1[:], cml[l][:], AL.add, AL.bypass)
            rf = T([B, OL[l]], F32, name=f"rf{l}")
            nc.vector.scalar_tensor_tensor(
                rf[:], iotaf_sb[:, CSEG[l]:CSEG[l] + OL[l]], Tg[l][:], cmg[:], AL.add, AL.subtract)
            r0 = T([B, OL[l]], F32, name=f"r0{l}")
            nc.vector.tensor_copy(r0[:], rf[:])
            nc.vector.copy_predicated(r0[:], m01[l][:], rt[:])
            pred = T([B, OL[l]], F32, name=f"pred{l}")
            nc.vector.tensor_scalar(pred[:], r0[:], float(KS[l]), None, AL.is_ge)
            ixf = T([B, OL[l]], F32, name=f"ixf{l}")
            nc.vector.scalar_tensor_tensor(ixf[:], pred[:], -8192.0, r0[:], AL.mult, AL.add)
            ixf2 = T([B, OL[l]], F32, name=f"ixf2_{l}")
            nc.vector.tensor_scalar_add(ixf2[:], ixf[:], float(CSEG[l]))
            nc.vector.tensor_copy(idx16[:, CSEG[l]:CSEG[l] + OL[l]], ixf2[:])

        dst16 = T([B, W_TOT], I16, name="dst16")
        if not (stage & 2):
            nc.vector.memset(dst16[:], 0)
        else:
            nc.gpsimd.local_scatter(dst16[:], iota16_sb[:], idx16[:],
                                    channels=128, num_elems=W_TOT, num_idxs=W_TOT)
        dstf = T([B, W_TOT], F32, name="dstf")
        nc.vector.tensor_copy(dstf[:], dst16[:])
        rs_in = T([B, W_TOT], F32, space="DRAM", name="rs_in")
        rs_out = T([BL, W_TOT], F32, space="DRAM", name="rs_out")
        nc.gpsimd.dma_start(rs_in[:], dstf[:])
        if stage & 4:
            nc.gpsimd.collective_compute(
                "ReduceScatter", AL.add, replica_groups=[list(range(NC))],
                ins=[rs_in[:]], outs=[rs_out[:]])
        else:
            nc.gpsimd.dma_start(rs_out[:], rs_in[0:BL, :])
        ids_sb = T([BL, W_TOT], F32, name="ids_sb")
        nc.gpsimd.dma_start(ids_sb[:], rs_out[:])
        ids32 = T([BL, W_TOT], I32, name="ids32")
        nc.vector.tensor_copy(ids32[:], ids_sb[:])
        nc.sync.dma_start(out_ids[:], ids32[:])

        nums = T([B, 3], F32, name="nums")
        for l in range(3):
            nc.vector.tensor_scalar_min(nums[:, l:l + 1], Tg[l][:], float(KS[l]))
        numi = T([B, 3], I32, name="numi")
        nc.vector.tensor_copy(numi[:], nums[:])
        nc.sync.dma_start(out_num[:], numi[:])
    nc.finalize()
    return nc


_PROG = {}


def _get_prog():
    import os
    stage = int(os.environ.get("KSTAGE", "7"))
    if stage not in _PROG:
        _PROG[stage] = _build_program(stage)
    return _PROG[stage]


def _build_C(ids, num, in_dim):
    b, s = ids.shape
    C = np.zeros((b, in_dim), np.float32)
    maskf = (np.arange(s)[None, :] < num).astype(np.float32)
    np.add.at(C, (np.arange(b)[:, None], ids.astype(np.int64)), maskf)
    return C


def _make_in_maps(inputs):
    w = [np.asarray(inputs["w0"], np.float32), np.asarray(inputs["w1"], np.float32),
         np.asarray(inputs["w2"], np.float32)]
    v = [np.asarray(inputs["v0"], np.float32), np.asarray(inputs["v1"], np.float32),
         np.asarray(inputs["v2"], np.float32)]
    C = [
        _build_C(np.asarray(inputs["inp_ids"]), np.asarray(inputs["inp_num"]), INS[0]),
        _build_C(np.asarray(inputs["s1_ids"]), np.asarray(inputs["s1_num"]), INS[1]),
        _build_C(np.asarray(inputs["s2_ids"]), np.asarray(inputs["s2_num"]), INS[2]),
    ]
    assert max(c.max() for c in C) < 256
    ct_u8 = [np.ascontiguousarray(c.T).astype(np.uint8) for c in C]
    in_maps = []
    for c in range(NC):
        m = {}
        for l in range(3):
            ol = OL[l]
            m[f"ct{l}"] = ct_u8[l]
            m[f"wt{l}"] = np.ascontiguousarray(w[l][c * ol:(c + 1) * ol, :].T)
            m[f"vc{l}"] = np.ascontiguousarray(v[l][:, c * ol:(c + 1) * ol])
        iota = np.concatenate([c * OL[l] + np.arange(OL[l]) for l in range(3)])
        m["iota16"] = np.broadcast_to(iota.astype(np.int16), (B, W_TOT)).copy()
        m["iotaf"] = np.broadcast_to(iota.astype(np.float32), (B, W_TOT)).copy()
        cmsk = np.zeros((B, NC), np.float32)
        cmsk[:, :c] = 1.0
        m["coremask"] = cmsk
        in_maps.append(m)
    return in_maps


def _assemble(r):
    ids = np.concatenate([r[c]["ids_out"] for c in range(NC)], axis=0).astype(np.int32)
    num3 = r[0]["num_out"].astype(np.int32)
    vns = [np.concatenate([r[c][f"vn{l}"] for c in range(NC)], axis=1) for l in range(3)]
    return (ids[:, 0:256], num3[:, 0:1], ids[:, 256:512], num3[:, 1:2],
            ids[:, 512:640], num3[:, 2:3], vns[0], vns[1], vns[2])


def _run(inputs, trace=False):
    in_maps = _make_in_maps(inputs)
    res = run_bass_kernel_spmd(_get_prog(), in_maps, list(range(NC)), trace=trace)
    out = _assemble(res.results)
    return out, res


def kernel(**inputs):
    out, _ = _run(inputs, trace=False)
    return out
